# revision 1
# baseline (speedup 1.0000x reference)
"""AGCNBlock kernel for 8 trn2 NeuronCores.

Data-parallel over the batch dim (B=8 -> 8 cores): each core holds one
graph ([N,N] adj slice + its nodes); GCN weights / w_b / tau replicated.
Shapes are fixed per the problem spec: B=8, N=2048, D=H=128, K=1434.

kernel(**inputs) takes FULL unsharded numpy inputs and returns the FULL
output tuple (out, H_out, new_adj, new_mask), matching the reference.
"""

import numpy as np

EPS = 1e-10
BIG = 1e10

B, N, D, HID, K = 8, 2048, 128, 128, 1434


def _one_graph_jnp(X, adj, mask, top_index, W1, b1, W2, b2, w_b0, tau):
    """Per-graph computation (jax). X:[N,D] adj:[N,N] mask:[N] top_index:[K]."""
    import jax
    import jax.numpy as jnp

    # --- stacked GCN layers: relu((adj @ x) @ W + b) ---
    h = jax.nn.relu(jnp.matmul(jnp.matmul(adj, X), W1) + b1)
    h = jax.nn.relu(jnp.matmul(jnp.matmul(adj, h), W2) + b2)
    hidden = mask[:, None] * h  # [N,H]

    # --- neighbor-softmax attention ---
    att_b = jnp.matmul(hidden, w_b0)[:, 0] + (mask - 1.0) * BIG  # [N]
    att_b = jnp.exp((att_b - jnp.max(att_b)) * jnp.abs(tau))
    denom = jnp.matmul(adj, att_b) + EPS  # [N]
    diag = jnp.diagonal(adj)  # [N]
    att_b = att_b * diag / denom
    att_b = att_b * jnp.sum(adj, axis=0)  # column sums (dnorm)
    att = att_b * mask

    Z = att[:, None] * hidden  # [N,H]

    # --- pooling via sampled top_index ---
    assign_m = adj[top_index, :]  # [K,N]
    assign_m = assign_m / (jnp.sum(assign_m, axis=0, keepdims=True) + EPS)
    H_out = jnp.matmul(assign_m, Z)  # [K,H]
    new_adj = jnp.matmul(jnp.matmul(assign_m, adj), assign_m.T)  # [K,K]

    # mean pool readout
    out = jnp.sum(mask[:, None] * hidden, axis=0) / jnp.sum(mask)  # [H]
    return out, H_out, new_adj


_PMAP_CACHE = {}


def _kernel_jax(X, adj, mask, W1, b1, W2, b2, w_b, tau, top_index):
    import jax

    n_dev = len(jax.devices())
    if "fn" not in _PMAP_CACHE:
        if n_dev >= B:
            fn = jax.pmap(
                _one_graph_jnp,
                in_axes=(0, 0, 0, 0, None, None, None, None, None, None),
                devices=jax.devices()[:B],
            )
        else:
            # fallback: vmap on a single device
            fn = jax.jit(
                jax.vmap(
                    _one_graph_jnp,
                    in_axes=(0, 0, 0, 0, None, None, None, None, None, None),
                )
            )
        _PMAP_CACHE["fn"] = fn
    fn = _PMAP_CACHE["fn"]

    ti = np.asarray(top_index).astype(np.int32)
    out, H_out, new_adj = fn(
        np.asarray(X, dtype=np.float32),
        np.asarray(adj, dtype=np.float32),
        np.asarray(mask, dtype=np.float32),
        ti,
        np.asarray(W1, dtype=np.float32),
        np.asarray(b1, dtype=np.float32),
        np.asarray(W2, dtype=np.float32),
        np.asarray(b2, dtype=np.float32),
        np.asarray(w_b, dtype=np.float32)[0],  # [H,1]
        np.float32(tau),
    )
    out = np.asarray(out, dtype=np.float32)
    H_out = np.asarray(H_out, dtype=np.float32)
    new_adj = np.asarray(new_adj, dtype=np.float32)
    new_mask = np.ones((B, K), dtype=np.float32)
    return out, H_out, new_adj, new_mask


def _kernel_numpy(X, adj, mask, W1, b1, W2, b2, w_b, tau, top_index):
    X = np.asarray(X, dtype=np.float32)
    adj = np.asarray(adj, dtype=np.float32)
    mask = np.asarray(mask, dtype=np.float32)
    W1 = np.asarray(W1, dtype=np.float32)
    b1 = np.asarray(b1, dtype=np.float32)
    W2 = np.asarray(W2, dtype=np.float32)
    b2 = np.asarray(b2, dtype=np.float32)
    w_b0 = np.asarray(w_b, dtype=np.float32)[0]  # [H,1]
    tau = np.float32(np.asarray(tau))
    ti = np.asarray(top_index).astype(np.int64)

    outs, Houts, nadjs = [], [], []
    for b in range(X.shape[0]):
        A, x, m, t = adj[b], X[b], mask[b], ti[b]
        h = np.maximum((A @ x) @ W1 + b1, 0.0)
        h = np.maximum((A @ h) @ W2 + b2, 0.0)
        hidden = m[:, None] * h
        att = (hidden @ w_b0)[:, 0] + (m - 1.0) * BIG
        att = np.exp((att - att.max()) * abs(tau))
        denom = A @ att + EPS
        att = att * np.diagonal(A) / denom
        att = att * A.sum(axis=0)
        att = att * m
        Z = att[:, None] * hidden
        asg = A[t, :]
        asg = asg / (asg.sum(axis=0, keepdims=True) + EPS)
        Houts.append(asg @ Z)
        nadjs.append((asg @ A) @ asg.T)
        outs.append(hidden.sum(axis=0) / m.sum())
    out = np.stack(outs).astype(np.float32)
    H_out = np.stack(Houts).astype(np.float32)
    new_adj = np.stack(nadjs).astype(np.float32)
    new_mask = np.ones((X.shape[0], ti.shape[1]), dtype=np.float32)
    return out, H_out, new_adj, new_mask


def kernel(**inputs):
    try:
        return _kernel_jax(**inputs)
    except Exception:
        return _kernel_numpy(**inputs)


# revision 2
# speedup vs baseline: 1.0040x; 1.0040x over previous
"""AGCNBlock kernel for 8 trn2 NeuronCores.

Data-parallel over the batch dim (B=8 -> 8 cores): each core holds one
graph ([N,N] adj slice + its nodes); GCN weights / w_b / tau replicated.
Shapes are fixed per the problem spec: B=8, N=2048, D=H=128, K=1434.

kernel(**inputs) takes FULL unsharded numpy inputs and returns the FULL
output tuple (out, H_out, new_adj, new_mask), matching the reference.
"""

import numpy as np

EPS = 1e-10
BIG = 1e10

B, N, D, HID, K = 8, 2048, 128, 128, 1434


def _one_graph_jnp(X, adj, mask, top_index, W1, b1, W2, b2, w_b0, tau):
    """Per-graph computation (jax). X:[N,D] adj:[N,N] mask:[N] top_index:[K]."""
    import jax
    import jax.numpy as jnp

    # --- stacked GCN layers: relu((adj @ x) @ W + b) ---
    h = jax.nn.relu(jnp.matmul(jnp.matmul(adj, X), W1) + b1)
    h = jax.nn.relu(jnp.matmul(jnp.matmul(adj, h), W2) + b2)
    hidden = mask[:, None] * h  # [N,H]

    # --- neighbor-softmax attention ---
    att_b = jnp.matmul(hidden, w_b0)[:, 0] + (mask - 1.0) * BIG  # [N]
    att_b = jnp.exp((att_b - jnp.max(att_b)) * jnp.abs(tau))
    denom = jnp.matmul(adj, att_b) + EPS  # [N]
    diag = jnp.diagonal(adj)  # [N]
    att_b = att_b * diag / denom
    att_b = att_b * jnp.sum(adj, axis=0)  # column sums (dnorm)
    att = att_b * mask

    Z = att[:, None] * hidden  # [N,H]

    # --- pooling via sampled top_index ---
    # bf16 for the heavy pooling matmuls (20.4 of 23.5 GFLOP): linear ops
    # with no exp downstream, error ~1e-3 vs the 2e-2 gate. The GCN /
    # attention path above stays fp32 (feeds exp with huge arg spread).
    def mm16(a, b):
        return jnp.matmul(
            a.astype(jnp.bfloat16),
            b.astype(jnp.bfloat16),
            preferred_element_type=jnp.float32,
        ).astype(jnp.float32)

    assign_m = adj[top_index, :]  # [K,N]
    assign_m = assign_m / (jnp.sum(assign_m, axis=0, keepdims=True) + EPS)
    H_out = mm16(assign_m, Z)  # [K,H]
    new_adj = mm16(mm16(assign_m, adj), assign_m.T)  # [K,K]

    # mean pool readout
    out = jnp.sum(mask[:, None] * hidden, axis=0) / jnp.sum(mask)  # [H]
    return out, H_out, new_adj


_PMAP_CACHE = {}


def _kernel_jax(X, adj, mask, W1, b1, W2, b2, w_b, tau, top_index):
    import jax

    n_dev = len(jax.devices())
    if "fn" not in _PMAP_CACHE:
        if n_dev >= B:
            fn = jax.pmap(
                _one_graph_jnp,
                in_axes=(0, 0, 0, 0, None, None, None, None, None, None),
                devices=jax.devices()[:B],
            )
        else:
            # fallback: vmap on a single device
            fn = jax.jit(
                jax.vmap(
                    _one_graph_jnp,
                    in_axes=(0, 0, 0, 0, None, None, None, None, None, None),
                )
            )
        _PMAP_CACHE["fn"] = fn
    fn = _PMAP_CACHE["fn"]

    ti = np.asarray(top_index).astype(np.int32)
    out, H_out, new_adj = fn(
        np.asarray(X, dtype=np.float32),
        np.asarray(adj, dtype=np.float32),
        np.asarray(mask, dtype=np.float32),
        ti,
        np.asarray(W1, dtype=np.float32),
        np.asarray(b1, dtype=np.float32),
        np.asarray(W2, dtype=np.float32),
        np.asarray(b2, dtype=np.float32),
        np.asarray(w_b, dtype=np.float32)[0],  # [H,1]
        np.float32(tau),
    )
    out = np.asarray(out, dtype=np.float32)
    H_out = np.asarray(H_out, dtype=np.float32)
    new_adj = np.asarray(new_adj, dtype=np.float32)
    new_mask = np.ones((B, K), dtype=np.float32)
    return out, H_out, new_adj, new_mask


def _kernel_numpy(X, adj, mask, W1, b1, W2, b2, w_b, tau, top_index):
    X = np.asarray(X, dtype=np.float32)
    adj = np.asarray(adj, dtype=np.float32)
    mask = np.asarray(mask, dtype=np.float32)
    W1 = np.asarray(W1, dtype=np.float32)
    b1 = np.asarray(b1, dtype=np.float32)
    W2 = np.asarray(W2, dtype=np.float32)
    b2 = np.asarray(b2, dtype=np.float32)
    w_b0 = np.asarray(w_b, dtype=np.float32)[0]  # [H,1]
    tau = np.float32(np.asarray(tau))
    ti = np.asarray(top_index).astype(np.int64)

    outs, Houts, nadjs = [], [], []
    for b in range(X.shape[0]):
        A, x, m, t = adj[b], X[b], mask[b], ti[b]
        h = np.maximum((A @ x) @ W1 + b1, 0.0)
        h = np.maximum((A @ h) @ W2 + b2, 0.0)
        hidden = m[:, None] * h
        att = (hidden @ w_b0)[:, 0] + (m - 1.0) * BIG
        att = np.exp((att - att.max()) * abs(tau))
        denom = A @ att + EPS
        att = att * np.diagonal(A) / denom
        att = att * A.sum(axis=0)
        att = att * m
        Z = att[:, None] * hidden
        asg = A[t, :]
        asg = asg / (asg.sum(axis=0, keepdims=True) + EPS)
        Houts.append(asg @ Z)
        nadjs.append((asg @ A) @ asg.T)
        outs.append(hidden.sum(axis=0) / m.sum())
    out = np.stack(outs).astype(np.float32)
    H_out = np.stack(Houts).astype(np.float32)
    new_adj = np.stack(nadjs).astype(np.float32)
    new_mask = np.ones((X.shape[0], ti.shape[1]), dtype=np.float32)
    return out, H_out, new_adj, new_mask


def kernel(**inputs):
    try:
        return _kernel_jax(**inputs)
    except Exception:
        return _kernel_numpy(**inputs)
